# revision 12
# baseline (speedup 1.0000x reference)
"""Trainium2 Bass kernel: MixedScore MultiHeadAttention (v3).

Math (per batch b, head h):
  S[r,c]   = (q[b,h,r,:] . k[b,h,c,:]) / 4
  t_m[r,c] = a_m*S + c_m*Q + b1_m          (Q = cost_mat[b])
  mixed    = sum_m w2_m * relu(t_m)  (+ b2, dropped: softmax shift-invariant)
  out      = softmax_c(mixed) @ v
Folding |w2_m| into (a_m, c_m, b1_m) gives mixed = sum_m s_m relu(A_m S + C_m Q + B_m),
s_m = sign(w2_m).

Layout (per core; core = (batch, half-of-heads) shard, 8 heads/core):
  - qhi (128, 8, 512) fp16: partitions 0:64 = S^T 64-c chunk (staged per
    head), 64:128 = cost^T (one batched DMA per buffer; 2 buffers alternate
    across heads).  S^T emitted 128-c-wide on PE -> fp16 stage copy ->
    SBUF->SBUF DMA split into the two qhi slots (SP + Pool SWDGE queues).
  - mix1 per chunk ci (128 c): 8 groups g; the two jj matmuls of a group
    (same (128,128) fp16 stationary; dedupe pass deletes the second
    LDWEIGHTS instruction) write one (128,2,512) 2-bank PSUM tile.
  - relu(+bias B_m) = ONE FD-1024 op per group tile, alternating ACT/DVE
    1:1 (measured 1089/1246 ns; ACT also owns exp+copies -> ~175us each),
    fp8e4m3 out into (128,2,512) r1 tiles.
  - mix2: one fp8 DoubleRow matmul per group (k-tiles = the jj strips,
    128-col sign stationary), lagged 3 groups, accumulating the full
    (128 c, 512 r) mixed^T chunk in one PSUM bank.
  - exp (ACT, FD-512 on 128 partitions) -> wx fp16 (|mixed| < 2.3 on this
    data, exp <= 9.3, fp16-safe); PV: single K=128 fp16 matmul per chunk,
    lhsT = [v | ones] (128,17); col 16 = softmax denominator; divide on
    host.  PV + out-copy deferred into the next chunk.
PSUM: ps1 3 x (128,2,512) [6 banks, shared by S^T blocks (bank 0 only) and
mix1 -> 3-deep PE runway; this depth is what keeps PE ~95% busy] + mix2
(128,512) + pvT (17,512) = 8 banks.
Measured levers: PE col-streaming (1 col of 128/cycle) is the hard floor:
mix1 262k + mix2 131k (DR halves k-passes) + S^T/PV 33k cols ~= 184us busy;
relu is the ACT/DVE floor (33.5M PSUM-fp32 elems at 1x, FD-1024 amortizes
the ~170-250 cycle access overhead).  tile_position 32x32 packing was tried
and is NOT faster here (issue-bound ~62ns/tile-MM; dense mix1 already
saturates column streaming).  TRN2 matmul cannot emit 16-bit PSUM (TRN3+
only), so relu stays 1x.  psmx single-buffering puts exp on the PE's FIFO
critical path -> mix2 lag 3 hides it.  Engines cannot write strided
partitions (BIR verifier) -- scatter via DMA instead.
HW exec: ~228.6us (vs 250us v1 baseline); rel err ~7.5e-3 (fp8 relu path).
Further measured dead ends: --enable-ldw-opt=true fails walrus codegen on
this BIR; DoubleRowSwInterleave mix2 weights neutral; S^T block-pairing and
merged q/k DMA both regress; mix2 lag 4 slightly worse than 3.  Beware the
P0 power-state downclock: after ~3h of sustained runs the same NEFF reads
~272us (all engines uniformly x1.185 = 2.4/2.0 GHz); it recovers on its
own -- don't chase phantom regressions while throttled.
"""

import itertools
import os
import sys

import ml_dtypes
import numpy as np

sys.path.insert(0, "/opt/trn_rl_repo")

import concourse.bass as bass  # noqa: E402
import concourse.mybir as mybir  # noqa: E402
from concourse import bacc, tile  # noqa: E402
from concourse.bass_utils import run_bass_kernel_spmd  # noqa: E402

FP = mybir.dt.float32
FPR = mybir.dt.float32r
F8 = mybir.dt.float8e4
BF16 = mybir.dt.bfloat16
FP16 = mybir.dt.float16
B, H, R, C, D, M = 4, 16, 512, 512, 16, 16
HPC = 8  # heads per core
NCORES = 8

AF = mybir.ActivationFunctionType
ALU = mybir.AluOpType
DR = mybir.MatmulPerfMode.DoubleRow
DRSI = mybir.MatmulPerfMode.DoubleRowSwInterleave

last_results = None  # BassKernelResults of the most recent run (for test.py)


def build_bass(w1_dt=FP16):
    nc = bacc.Bacc(None, target_bir_lowering=False, debug=False)

    qT = nc.declare_dram_parameter("qT", [D, HPC, R], FP16, isOutput=False)
    kT = nc.declare_dram_parameter("kT", [D, HPC, C], FP16, isOutput=False)
    costT = nc.declare_dram_parameter("costT", [64, 8, R], FP16, isOutput=False)
    vxp = nc.declare_dram_parameter("vxp", [128, HPC, 4, 17], FP16, isOutput=False)
    w1s = nc.declare_dram_parameter("w1s", [128, HPC, 8, 128], w1_dt, isOutput=False)
    w2s = nc.declare_dram_parameter("w2s", [128, HPC, 8, 2, 128], F8, isOutput=False)
    bvs = nc.declare_dram_parameter("bvs", [128, HPC], FP, isOutput=False)
    outp = nc.declare_dram_parameter("out", [HPC, D + 1, R], FP, isOutput=True)

    with tile.TileContext(nc) as tc:
        with (
            tc.tile_pool(name="const", bufs=1) as constp,
            tc.tile_pool(name="qhi", bufs=1) as qhip,
            tc.tile_pool(name="r1", bufs=12) as r1p,
            tc.tile_pool(name="wexp", bufs=8) as wexpp,
            tc.tile_pool(name="osb", bufs=4) as osbp,
            tc.tile_pool(name="stg", bufs=6) as stgp,
            tc.tile_pool(name="ps1", bufs=3, space="PSUM") as ps1p,
            tc.tile_pool(name="psmx", bufs=1, space="PSUM") as psmxp,
            tc.tile_pool(name="pspv", bufs=1, space="PSUM") as pspvp,
        ):
            w1_sb = constp.tile([128, HPC, 8, 128], w1_dt)
            w2_sb = constp.tile([128, HPC, 8, 2, 128], F8)
            bv_sb = constp.tile([128, HPC], FP)
            qT_sb = constp.tile([D, HPC, R], FP16)
            kT_sb = constp.tile([D, HPC, C], FP16)
            vx_sb = constp.tile([128, HPC, 4, 17], FP16)

            qhi = [qhip.tile([128, 8, 512], FP16, name=f"qhi{i}", tag=f"qhi{i}") for i in range(2)]
            # Preamble DMAs ordered by first use, split across the two HWDGE
            # queues (SP carries cost/qhi, ACT carries weights) so head 0 can
            # start within ~2us instead of waiting on a serial 6 MB preamble.
            # ACT issues only what head 0 needs (its engine program must be
            # free early for the head-0 S^T copies + first relus); everything
            # else streams on the idle SP queue in need-order.
            # the scalar-queue HWDGE starts moving payload ~6us before the
            # SP queue (measured 2.5 vs 8.3us); head 0's critical S^T inputs
            # ride it, the weights it used to carry go to SP instead.
            nc.scalar.dma_start(out=qT_sb[:, 0], in_=qT[:, 0])
            nc.scalar.dma_start(out=kT_sb[:, 0], in_=kT[:, 0])
            nc.sync.dma_start(out=w1_sb[:, 0], in_=w1s[:, 0])
            nc.sync.dma_start(out=bv_sb[:], in_=bvs[:])
            nc.scalar.dma_start(out=w2_sb[:, 0], in_=w2s[:, 0])
            nc.sync.dma_start(out=qhi[0][64:128, 0:2, :], in_=costT[:, 0:2, :])
            nc.sync.dma_start(out=qhi[0][64:128, 2:, :], in_=costT[:, 2:, :])
            nc.scalar.dma_start(out=vx_sb[:, 0], in_=vxp[:, 0])
            nc.sync.dma_start(out=qT_sb[:, 1:], in_=qT[:, 1:])
            nc.sync.dma_start(out=kT_sb[:, 1:], in_=kT[:, 1:])
            nc.sync.dma_start(out=w1_sb[:, 1], in_=w1s[:, 1])
            nc.sync.dma_start(out=w2_sb[:, 1], in_=w2s[:, 1])
            nc.sync.dma_start(out=vx_sb[:, 1], in_=vxp[:, 1])
            nc.sync.dma_start(out=qhi[1][64:128, :, :], in_=costT[:])
            nc.sync.dma_start(out=w1_sb[:, 2:], in_=w1s[:, 2:])
            nc.sync.dma_start(out=w2_sb[:, 2:], in_=w2s[:, 2:])
            nc.sync.dma_start(out=vx_sb[:, 2:], in_=vxp[:, 2:])

            # 1:1: DVE relu 1246ns vs ACT 1089ns, but ACT also owns exp
            # (20us) + copies (14us) -- strict alternation lands ~173us each.
            relu_rr = itertools.cycle([nc.vector, nc.scalar])
            stage_rr = itertools.cycle([nc.vector, nc.scalar])
            dma_rr = itertools.cycle([nc.sync, nc.gpsimd])

            def emit_st1(hs, jb):
                qdst = qhi[hs % 2]
                pst = ps1p.tile([128, 2, 512], FP, name="p1", tag="p1")
                nc.tensor.matmul(
                    pst[:, 0, :],
                    lhsT=kT_sb[:, hs, 128 * jb : 128 * jb + 128],
                    rhs=qT_sb[:, hs, :],
                    start=True,
                    stop=True,
                )
                if hs == 0:
                    nc.vector.tensor_copy(
                        out=qdst[0:64, 2 * jb, :], in_=pst[0:64, 0, :]
                    )
                    nc.scalar.copy(
                        out=qdst[0:64, 2 * jb + 1, :], in_=pst[64:128, 0, :]
                    )
                    return
                stg = stgp.tile([128, 512], FP16, name="stg", tag="stg")
                eng = next(stage_rr)
                if eng is nc.scalar:
                    nc.scalar.copy(out=stg[:], in_=pst[:, 0, :])
                else:
                    eng.tensor_copy(out=stg[:], in_=pst[:, 0, :])
                for half in range(2):
                    next(dma_rr).dma_start(
                        out=qdst[0:64, 2 * jb + half, :],
                        in_=stg[64 * half : 64 * half + 64, :],
                    )

            def emit_st2(hs, cp):
                # two 128-c-wide S^T blocks (chunk pair cp) into one 2-bank
                # PSUM tile: both banks used, back-to-back MMs hide the
                # drain, and mix1's 3-deep runway is disturbed half as often.
                qdst = qhi[hs % 2]
                pst = ps1p.tile([128, 2, 512], FP, name="p1", tag="p1")
                for i in range(2):
                    jb = 2 * cp + i
                    nc.tensor.matmul(
                        pst[:, i, :],
                        lhsT=kT_sb[:, hs, 128 * jb : 128 * jb + 128],
                        rhs=qT_sb[:, hs, :],
                        start=True,
                        stop=True,
                    )
                if hs == 0:
                    for i in range(2):
                        jb = 2 * cp + i
                        nc.vector.tensor_copy(
                            out=qdst[0:64, 2 * jb, :], in_=pst[0:64, i, :]
                        )
                        nc.scalar.copy(
                            out=qdst[0:64, 2 * jb + 1, :], in_=pst[64:128, i, :]
                        )
                    return
                for i in range(2):
                    jb = 2 * cp + i
                    stg = stgp.tile([128, 512], FP16, name="stg", tag="stg")
                    eng = next(stage_rr)
                    if eng is nc.scalar:
                        nc.scalar.copy(out=stg[:], in_=pst[:, i, :])
                    else:
                        eng.tensor_copy(out=stg[:], in_=pst[:, i, :])
                    for half in range(2):
                        next(dma_rr).dma_start(
                            out=qdst[0:64, 2 * jb + half, :],
                            in_=stg[64 * half : 64 * half + 64, :],
                        )

            def emit_relu(r1slot, p1ap, hh):
                eng = next(relu_rr)
                if eng is nc.scalar:
                    nc.scalar.activation(
                        r1slot, p1ap, AF.Relu, bias=bv_sb[:, hh : hh + 1]
                    )
                else:
                    eng.tensor_scalar(
                        out=r1slot,
                        in0=p1ap,
                        scalar1=bv_sb[:, hh : hh + 1],
                        scalar2=0.0,
                        op0=ALU.add,
                        op1=ALU.max,
                    )

            for jb in range(4):
                emit_st1(0, jb)

            # PV matmuls for chunk ci are deferred into chunk ci+1 so the PE
            # never waits on the exp; the head's out-copy rides the last flush.
            hold = {"pend": None, "pvT": None}

            def flush_pv():
                if hold["pend"] is None:
                    return
                hh_, ci_, wx_ = hold["pend"]
                hold["pend"] = None
                if ci_ == 0:
                    hold["pvT"] = pspvp.tile([17, 512], FP, name="pvT", tag="pvT")
                pvT = hold["pvT"]
                nc.tensor.matmul(
                    pvT[:],
                    lhsT=vx_sb[:, hh_, ci_, :],
                    rhs=wx_[:],
                    start=(ci_ == 0),
                    stop=(ci_ == 3),
                )
                if ci_ == 3:
                    ot = osbp.tile([17, 512], FP, name="ot", tag="ot")
                    if hh_ % 2 == 0 or hh_ == HPC - 1:
                        nc.vector.tensor_copy(out=ot[:], in_=pvT[:])
                    else:
                        nc.scalar.copy(out=ot[:], in_=pvT[:])
                    nc.sync.dma_start(out=outp[hh_], in_=ot[:])

            for hh in range(HPC):
                qh = qhi[hh % 2]
                for ci in range(4):
                    lag = 1 if (hh == HPC - 1 and ci == 3) else 3
                    if hh + 1 < HPC:
                        emit_st1(hh + 1, ci)
                    pmx = psmxp.tile([128, 512], FP)
                    r1t = [None] * 8

                    def emit_mix2(g):
                        # DR k-tiles = the two jj strips of group g; 128-col
                        # stationary accumulates the full (128 c, 512 r) chunk
                        nc.tensor.matmul(
                            pmx[:],
                            lhsT=w2_sb[:, hh, g, :, :],
                            rhs=r1t[g][:],
                            start=(g == 0),
                            stop=(g == 7),
                            perf_mode=DR,
                        )

                    # mix1: both jj matmuls of a group write one 2-bank PSUM
                    # tile (banks = jj) so the scheduler keeps the pair
                    # adjacent and the dedupe deletes the repeat LDWEIGHTS;
                    # relu drains FD-1024 per group; mix2 lags one group.
                    for g in range(8):
                        r1g = r1p.tile([128, 2, 512], F8, name="r1", tag="r1")
                        p1g = ps1p.tile([128, 2, 512], FP, name="p1", tag="p1")
                        for jj in range(2):
                            nc.tensor.matmul(
                                p1g[:, jj, :],
                                lhsT=w1_sb[:, hh, g, :],
                                rhs=qh[:, 2 * ci + jj, :],
                                start=True,
                                stop=True,
                            )
                        emit_relu(r1g[:], p1g[:], hh)
                        r1t[g] = r1g
                        if g == 1:
                            flush_pv()
                        if g >= lag:
                            emit_mix2(g - lag)
                    for gg in range(8 - lag, 8):
                        emit_mix2(gg)
                    wx = wexpp.tile([128, 512], FP16, name="wx", tag="wexp")
                    nc.scalar.activation(wx[:], pmx[:], AF.Exp)
                    hold["pend"] = (hh, ci, wx)
            flush_pv()
    if not int(os.environ.get("KERNEL_NO_DEDUPE", "0")):
        _dedupe_weight_loads(nc)
    nc.finalize()
    return nc


def _dedupe_weight_loads(nc):
    """The tile framework emits an explicit InstLdweights before every
    matmul; the PE array keeps its stationary until the next load, so a
    back-to-back reload of the identical AP is dead -- delete it (only when
    it carries no sync info; the first load of a run is the one that waits
    on the weight DMA and is never a duplicate)."""
    n = 0
    for bb in nc.m.functions[0].blocks:
        last = None
        keep = []
        for ins in bb.instructions:
            if isinstance(ins, mybir.InstLdweights):
                w = ins.ins[0]
                key = (w.memref, w.offset, str(w.ap), str(w.dtype),
                       ins.tile_position, ins.tile_size, ins.perf_mode)
                si = ins.sync_info
                clean = si is None or (not si.on_wait and not si.on_update)
                if key == last and clean:
                    n += 1
                    continue
                last = key
            keep.append(ins)
        if len(keep) != len(bb.instructions):
            bb.instructions = keep
    print(f"deduped {n} weight loads", file=sys.stderr)


def prepare_in_maps(q, k, v, cost_mat, mix1_weight, mix1_bias, mix2_weight, mix2_bias):
    q = np.asarray(q, np.float32)
    k = np.asarray(k, np.float32)
    v = np.asarray(v, np.float32)
    cost_mat = np.asarray(cost_mat, np.float32)
    mix1_weight = np.asarray(mix1_weight, np.float32)
    mix1_bias = np.asarray(mix1_bias, np.float32)
    mix2_weight = np.asarray(mix2_weight, np.float32)
    mix2_bias = np.asarray(mix2_bias, np.float32)

    in_maps = []
    for core in range(NCORES):
        b = core // 2
        h0 = (core % 2) * HPC
        qT = (np.ascontiguousarray(q[b, h0 : h0 + HPC].transpose(2, 0, 1)) * 0.25).astype(np.float16)
        kT = np.ascontiguousarray(k[b, h0 : h0 + HPC].transpose(2, 0, 1)).astype(np.float16)
        costT = np.ascontiguousarray(cost_mat[b].T.reshape(8, 64, R).transpose(1, 0, 2)).astype(np.float16)
        vv = v[b, h0 : h0 + HPC]  # (HPC, C, D)
        vxa = np.empty((128, HPC, 4, 17), np.float32)
        vxa[:, :, :, :16] = vv.reshape(HPC, 4, 128, 16).transpose(2, 0, 1, 3)
        vxa[:, :, :, 16] = 1.0
        vxa = vxa.astype(np.float16)

        w1 = mix1_weight[h0 : h0 + HPC]  # (HPC, 2, M)
        b1 = mix1_bias[h0 : h0 + HPC]  # (HPC, M)
        w2 = mix2_weight[h0 : h0 + HPC, :, 0]  # (HPC, M)
        aw = np.abs(w2)
        sg = np.sign(w2).astype(np.float32)
        A = (w1[:, 0, :] * aw).astype(np.float32)  # (HPC, M)
        Cc = (w1[:, 1, :] * aw).astype(np.float32)
        Bb = (b1 * aw).astype(np.float32)

        w1s = np.zeros((128, HPC, 8, 128), np.float32)
        for g in range(8):
            for c8 in range(8):
                cols = slice(c8 * 16, c8 * 16 + 16)
                w1s[8 * g + c8, :, g, cols] = A
                w1s[64 + 8 * g + c8, :, g, cols] = Cc
        w1s = w1s.astype(np.float16)
        # w2s[16 c8 + m, h, g, jj, 64 jj + 8 g + c8] = sign(w2_m):
        # DR k-tile jj of group g feeds chunk column 64 jj + 8 g + c8 of the
        # (128 c, 512 r) mixed output.
        w2s = np.zeros((128, HPC, 8, 2, 128), np.float32)
        for g in range(8):
            for jj in range(2):
                for c8 in range(8):
                    w2s[c8 * 16 : c8 * 16 + 16, :, g, jj, 64 * jj + 8 * g + c8] = sg.T
        w2s = w2s.astype(ml_dtypes.float8_e4m3)
        bvs = np.tile(Bb.T, (8, 1)).astype(np.float32)  # (128, HPC)

        in_maps.append(
            dict(qT=qT, kT=kT, costT=costT, vxp=vxa, w1s=w1s, w2s=w2s, bvs=bvs)
        )
    return in_maps


def assemble(results):
    full = np.empty((B, R, H * D), np.float32)
    for core in range(NCORES):
        b = core // 2
        c0 = (core % 2) * HPC * D
        o = results[core]["out"]  # (HPC, D+1, R); row D is the softmax denom
        o = o[:, :D, :] / o[:, D : D + 1, :]
        full[b, :, c0 : c0 + HPC * D] = o.transpose(2, 0, 1).reshape(R, HPC * D)
    return full


_nc_cache = None


def _install_ntff_hook():
    """The agent image's antenv lacks axon_hooks; recreate it and register
    the ctypes NTFF profiling hook so trace=True yields exec times."""
    import types

    try:
        import antenv

        try:
            import antenv.axon_hooks  # noqa: F401

            return
        except ImportError:
            pass
        mod = types.ModuleType("antenv.axon_hooks")
        mod._hook = None
        mod.set_axon_ntff_profile_hook = lambda h: setattr(mod, "_hook", h)
        mod.get_axon_ntff_profile_hook = lambda: mod._hook
        sys.modules["antenv.axon_hooks"] = mod
        antenv.axon_hooks = mod
        from trn_agent_boot.trn_boot import _ntff_profile_via_ctypes

        mod._hook = _ntff_profile_via_ctypes("/opt/axon/libaxon_pjrt.so")
    except Exception as e:  # profiling is best-effort
        print(f"ntff hook install failed: {e}", file=sys.stderr)


def kernel(**inputs) -> np.ndarray:
    global _nc_cache, last_results
    if _nc_cache is None:
        _nc_cache = build_bass()
    in_maps = prepare_in_maps(**inputs)
    trace = bool(int(os.environ.get("KERNEL_TRACE", "0")))
    if trace:
        _install_ntff_hook()
        import concourse.bass_utils as bu

        bu.upload_artifacts = lambda tmpdir: f"local:{tmpdir}"
    res = run_bass_kernel_spmd(_nc_cache, in_maps, list(range(NCORES)), trace=trace)
    last_results = res
    return assemble(res.results)



# revision 13
# speedup vs baseline: 1.0878x; 1.0878x over previous
"""Trainium2 Bass kernel: MixedScore MultiHeadAttention (v3).

Math (per batch b, head h):
  S[r,c]   = (q[b,h,r,:] . k[b,h,c,:]) / 4
  t_m[r,c] = a_m*S + c_m*Q + b1_m          (Q = cost_mat[b])
  mixed    = sum_m w2_m * relu(t_m)  (+ b2, dropped: softmax shift-invariant)
  out      = softmax_c(mixed) @ v
Folding |w2_m| into (a_m, c_m, b1_m) gives mixed = sum_m s_m relu(A_m S + C_m Q + B_m),
s_m = sign(w2_m).

Layout (per core; core = (batch, half-of-heads) shard, 8 heads/core):
  - qhi (128, 8, 512) fp16: partitions 0:64 = S^T 64-c chunk (staged per
    head), 64:128 = cost^T (one batched DMA per buffer; 2 buffers alternate
    across heads).  S^T emitted 128-c-wide on PE -> fp16 stage copy ->
    SBUF->SBUF DMA split into the two qhi slots (SP + Pool SWDGE queues).
  - mix1 per chunk ci (128 c): 8 groups g; the two jj matmuls of a group
    (same (128,128) fp16 stationary; dedupe pass deletes the second
    LDWEIGHTS instruction) write one (128,2,512) 2-bank PSUM tile.
  - relu(+bias B_m) = ONE FD-1024 op per group tile, alternating ACT/DVE
    1:1 (measured 1089/1246 ns; ACT also owns exp+copies -> ~175us each),
    fp8e4m3 out into (128,2,512) r1 tiles.
  - mix2: one fp8 DoubleRow matmul per group (k-tiles = the jj strips,
    128-col sign stationary), lagged 3 groups, accumulating the full
    (128 c, 512 r) mixed^T chunk in one PSUM bank.
  - exp (ACT, FD-512 on 128 partitions) -> wx fp16 (|mixed| < 2.3 on this
    data, exp <= 9.3, fp16-safe); PV: single K=128 fp16 matmul per chunk,
    lhsT = [v | ones] (128,17); col 16 = softmax denominator; divide on
    host.  PV + out-copy deferred into the next chunk.
PSUM: ps1 3 x (128,2,512) [6 banks, shared by S^T blocks (bank 0 only) and
mix1 -> 3-deep PE runway; this depth is what keeps PE ~95% busy] + mix2
(128,512) + pvT (17,512) = 8 banks.
Measured levers: PE col-streaming (1 col of 128/cycle) is the hard floor:
mix1 262k + mix2 131k (DR halves k-passes) + S^T/PV 33k cols ~= 184us busy;
relu is the ACT/DVE floor (33.5M PSUM-fp32 elems at 1x, FD-1024 amortizes
the ~170-250 cycle access overhead).  tile_position 32x32 packing was tried
and is NOT faster here (issue-bound ~62ns/tile-MM; dense mix1 already
saturates column streaming).  TRN2 matmul cannot emit 16-bit PSUM (TRN3+
only), so relu stays 1x.  psmx single-buffering puts exp on the PE's FIFO
critical path -> mix2 lag 3 hides it.  Engines cannot write strided
partitions (BIR verifier) -- scatter via DMA instead.
HW exec: ~228.6us (vs 250us v1 baseline); rel err ~7.5e-3 (fp8 relu path).
Further measured dead ends: --enable-ldw-opt=true fails walrus codegen on
this BIR; DoubleRowSwInterleave mix2 weights neutral; S^T block-pairing and
merged q/k DMA both regress; mix2 lag 4 slightly worse than 3.  Beware the
P0 power-state downclock: after ~3h of sustained runs the same NEFF reads
~272us (all engines uniformly x1.185 = 2.4/2.0 GHz); it recovers on its
own -- don't chase phantom regressions while throttled.
"""

import itertools
import os
import sys

import ml_dtypes
import numpy as np

sys.path.insert(0, "/opt/trn_rl_repo")

import concourse.bass as bass  # noqa: E402
import concourse.mybir as mybir  # noqa: E402
from concourse import bacc, tile  # noqa: E402
from concourse.bass_utils import run_bass_kernel_spmd  # noqa: E402

FP = mybir.dt.float32
FPR = mybir.dt.float32r
F8 = mybir.dt.float8e4
BF16 = mybir.dt.bfloat16
FP16 = mybir.dt.float16
B, H, R, C, D, M = 4, 16, 512, 512, 16, 16
HPC = 8  # heads per core
NCORES = 8

AF = mybir.ActivationFunctionType
ALU = mybir.AluOpType
DR = mybir.MatmulPerfMode.DoubleRow
DRSI = mybir.MatmulPerfMode.DoubleRowSwInterleave

last_results = None  # BassKernelResults of the most recent run (for test.py)


def build_bass(w1_dt=FP16):
    nc = bacc.Bacc(None, target_bir_lowering=False, debug=False)

    qT = nc.declare_dram_parameter("qT", [D, HPC, R], FP16, isOutput=False)
    kT = nc.declare_dram_parameter("kT", [D, HPC, C], FP16, isOutput=False)
    costT = nc.declare_dram_parameter("costT", [64, 8, R], FP16, isOutput=False)
    vxp = nc.declare_dram_parameter("vxp", [128, HPC, 4, 17], FP16, isOutput=False)
    w1s = nc.declare_dram_parameter("w1s", [128, HPC, 8, 128], w1_dt, isOutput=False)
    w2s = nc.declare_dram_parameter("w2s", [128, HPC, 8, 2, 128], F8, isOutput=False)
    bvs = nc.declare_dram_parameter("bvs", [128, HPC], FP, isOutput=False)
    outp = nc.declare_dram_parameter("out", [HPC, D + 1, R], FP, isOutput=True)

    with tile.TileContext(nc) as tc:
        with (
            tc.tile_pool(name="const", bufs=1) as constp,
            tc.tile_pool(name="qhi", bufs=1) as qhip,
            tc.tile_pool(name="r1", bufs=12) as r1p,
            tc.tile_pool(name="wexp", bufs=8) as wexpp,
            tc.tile_pool(name="osb", bufs=4) as osbp,
            tc.tile_pool(name="stg", bufs=6) as stgp,
            tc.tile_pool(name="ps1", bufs=3, space="PSUM") as ps1p,
            tc.tile_pool(name="psmx", bufs=1, space="PSUM") as psmxp,
            tc.tile_pool(name="pspv", bufs=1, space="PSUM") as pspvp,
        ):
            w1_sb = constp.tile([128, HPC, 8, 128], w1_dt)
            w2_sb = constp.tile([128, HPC, 8, 2, 128], F8)
            bv_sb = constp.tile([128, HPC], FP)
            qT_sb = constp.tile([D, HPC, R], FP16)
            kT_sb = constp.tile([D, HPC, C], FP16)
            vx_sb = constp.tile([128, HPC, 4, 17], FP16)

            qhi = [qhip.tile([128, 8, 512], FP16, name=f"qhi{i}", tag=f"qhi{i}") for i in range(2)]
            # Preamble DMAs ordered by first use, split across the two HWDGE
            # queues (SP carries cost/qhi, ACT carries weights) so head 0 can
            # start within ~2us instead of waiting on a serial 6 MB preamble.
            # ACT issues only what head 0 needs (its engine program must be
            # free early for the head-0 S^T copies + first relus); everything
            # else streams on the idle SP queue in need-order.
            nc.sync.dma_start(out=qT_sb[:, 0], in_=qT[:, 0])
            nc.sync.dma_start(out=kT_sb[:, 0], in_=kT[:, 0])
            nc.scalar.dma_start(out=w1_sb[:, 0], in_=w1s[:, 0])
            nc.scalar.dma_start(out=bv_sb[:], in_=bvs[:])
            nc.scalar.dma_start(out=w2_sb[:, 0], in_=w2s[:, 0])
            nc.sync.dma_start(out=qhi[0][64:128, 0:2, :], in_=costT[:, 0:2, :])
            nc.sync.dma_start(out=qhi[0][64:128, 2:, :], in_=costT[:, 2:, :])
            nc.scalar.dma_start(out=vx_sb[:, 0], in_=vxp[:, 0])
            nc.sync.dma_start(out=qT_sb[:, 1:], in_=qT[:, 1:])
            nc.sync.dma_start(out=kT_sb[:, 1:], in_=kT[:, 1:])
            nc.sync.dma_start(out=w1_sb[:, 1], in_=w1s[:, 1])
            nc.sync.dma_start(out=w2_sb[:, 1], in_=w2s[:, 1])
            nc.sync.dma_start(out=vx_sb[:, 1], in_=vxp[:, 1])
            nc.sync.dma_start(out=qhi[1][64:128, :, :], in_=costT[:])
            nc.sync.dma_start(out=w1_sb[:, 2:], in_=w1s[:, 2:])
            nc.sync.dma_start(out=w2_sb[:, 2:], in_=w2s[:, 2:])
            nc.sync.dma_start(out=vx_sb[:, 2:], in_=vxp[:, 2:])

            # 1:1: DVE relu 1246ns vs ACT 1089ns, but ACT also owns exp
            # (20us) + copies (14us) -- strict alternation lands ~173us each.
            relu_rr = itertools.cycle([nc.vector, nc.scalar])
            stage_rr = itertools.cycle([nc.vector, nc.scalar])
            dma_rr = itertools.cycle([nc.sync, nc.gpsimd])

            def emit_st1(hs, jb):
                qdst = qhi[hs % 2]
                pst = ps1p.tile([128, 2, 512], FP, name="p1", tag="p1")
                nc.tensor.matmul(
                    pst[:, 0, :],
                    lhsT=kT_sb[:, hs, 128 * jb : 128 * jb + 128],
                    rhs=qT_sb[:, hs, :],
                    start=True,
                    stop=True,
                )
                if hs == 0:
                    nc.vector.tensor_copy(
                        out=qdst[0:64, 2 * jb, :], in_=pst[0:64, 0, :]
                    )
                    nc.scalar.copy(
                        out=qdst[0:64, 2 * jb + 1, :], in_=pst[64:128, 0, :]
                    )
                    return
                stg = stgp.tile([128, 512], FP16, name="stg", tag="stg")
                eng = next(stage_rr)
                if eng is nc.scalar:
                    nc.scalar.copy(out=stg[:], in_=pst[:, 0, :])
                else:
                    eng.tensor_copy(out=stg[:], in_=pst[:, 0, :])
                for half in range(2):
                    next(dma_rr).dma_start(
                        out=qdst[0:64, 2 * jb + half, :],
                        in_=stg[64 * half : 64 * half + 64, :],
                    )

            def emit_st2(hs, cp):
                # two 128-c-wide S^T blocks (chunk pair cp) into one 2-bank
                # PSUM tile: both banks used, back-to-back MMs hide the
                # drain, and mix1's 3-deep runway is disturbed half as often.
                qdst = qhi[hs % 2]
                pst = ps1p.tile([128, 2, 512], FP, name="p1", tag="p1")
                for i in range(2):
                    jb = 2 * cp + i
                    nc.tensor.matmul(
                        pst[:, i, :],
                        lhsT=kT_sb[:, hs, 128 * jb : 128 * jb + 128],
                        rhs=qT_sb[:, hs, :],
                        start=True,
                        stop=True,
                    )
                if hs == 0:
                    for i in range(2):
                        jb = 2 * cp + i
                        nc.vector.tensor_copy(
                            out=qdst[0:64, 2 * jb, :], in_=pst[0:64, i, :]
                        )
                        nc.scalar.copy(
                            out=qdst[0:64, 2 * jb + 1, :], in_=pst[64:128, i, :]
                        )
                    return
                for i in range(2):
                    jb = 2 * cp + i
                    stg = stgp.tile([128, 512], FP16, name="stg", tag="stg")
                    eng = next(stage_rr)
                    if eng is nc.scalar:
                        nc.scalar.copy(out=stg[:], in_=pst[:, i, :])
                    else:
                        eng.tensor_copy(out=stg[:], in_=pst[:, i, :])
                    for half in range(2):
                        next(dma_rr).dma_start(
                            out=qdst[0:64, 2 * jb + half, :],
                            in_=stg[64 * half : 64 * half + 64, :],
                        )

            def emit_relu(r1slot, p1ap, hh):
                eng = next(relu_rr)
                if eng is nc.scalar:
                    nc.scalar.activation(
                        r1slot, p1ap, AF.Relu, bias=bv_sb[:, hh : hh + 1]
                    )
                else:
                    eng.tensor_scalar(
                        out=r1slot,
                        in0=p1ap,
                        scalar1=bv_sb[:, hh : hh + 1],
                        scalar2=0.0,
                        op0=ALU.add,
                        op1=ALU.max,
                    )

            for jb in range(4):
                emit_st1(0, jb)

            # PV matmuls for chunk ci are deferred into chunk ci+1 so the PE
            # never waits on the exp; the head's out-copy rides the last flush.
            hold = {"pend": None, "pvT": None}

            def flush_pv():
                if hold["pend"] is None:
                    return
                hh_, ci_, wx_ = hold["pend"]
                hold["pend"] = None
                if ci_ == 0:
                    hold["pvT"] = pspvp.tile([17, 512], FP, name="pvT", tag="pvT")
                pvT = hold["pvT"]
                nc.tensor.matmul(
                    pvT[:],
                    lhsT=vx_sb[:, hh_, ci_, :],
                    rhs=wx_[:],
                    start=(ci_ == 0),
                    stop=(ci_ == 3),
                )
                if ci_ == 3:
                    ot = osbp.tile([17, 512], FP, name="ot", tag="ot")
                    if hh_ % 2 == 0 or hh_ == HPC - 1:
                        nc.vector.tensor_copy(out=ot[:], in_=pvT[:])
                    else:
                        nc.scalar.copy(out=ot[:], in_=pvT[:])
                    nc.sync.dma_start(out=outp[hh_], in_=ot[:])

            for hh in range(HPC):
                qh = qhi[hh % 2]
                for ci in range(4):
                    lag = 1 if (hh == HPC - 1 and ci == 3) else 3
                    if hh + 1 < HPC:
                        emit_st1(hh + 1, ci)
                    pmx = psmxp.tile([128, 512], FP)
                    r1t = [None] * 8

                    def emit_mix2(g):
                        # DR k-tiles = the two jj strips of group g; 128-col
                        # stationary accumulates the full (128 c, 512 r) chunk
                        nc.tensor.matmul(
                            pmx[:],
                            lhsT=w2_sb[:, hh, g, :, :],
                            rhs=r1t[g][:],
                            start=(g == 0),
                            stop=(g == 7),
                            perf_mode=DR,
                        )

                    # mix1: both jj matmuls of a group write one 2-bank PSUM
                    # tile (banks = jj) so the scheduler keeps the pair
                    # adjacent and the dedupe deletes the repeat LDWEIGHTS;
                    # relu drains FD-1024 per group; mix2 lags one group.
                    for g in range(8):
                        r1g = r1p.tile([128, 2, 512], F8, name="r1", tag="r1")
                        p1g = ps1p.tile([128, 2, 512], FP, name="p1", tag="p1")
                        for jj in range(2):
                            nc.tensor.matmul(
                                p1g[:, jj, :],
                                lhsT=w1_sb[:, hh, g, :],
                                rhs=qh[:, 2 * ci + jj, :],
                                start=True,
                                stop=True,
                            )
                        emit_relu(r1g[:], p1g[:], hh)
                        r1t[g] = r1g
                        if g == 1:
                            flush_pv()
                        if g >= lag:
                            emit_mix2(g - lag)
                    for gg in range(8 - lag, 8):
                        emit_mix2(gg)
                    wx = wexpp.tile([128, 512], FP16, name="wx", tag="wexp")
                    nc.scalar.activation(wx[:], pmx[:], AF.Exp)
                    hold["pend"] = (hh, ci, wx)
            flush_pv()
    if not int(os.environ.get("KERNEL_NO_DEDUPE", "0")):
        _dedupe_weight_loads(nc)
    nc.finalize()
    return nc


def _dedupe_weight_loads(nc):
    """The tile framework emits an explicit InstLdweights before every
    matmul; the PE array keeps its stationary until the next load, so a
    back-to-back reload of the identical AP is dead -- delete it (only when
    it carries no sync info; the first load of a run is the one that waits
    on the weight DMA and is never a duplicate)."""
    n = 0
    for bb in nc.m.functions[0].blocks:
        last = None
        keep = []
        for ins in bb.instructions:
            if isinstance(ins, mybir.InstLdweights):
                w = ins.ins[0]
                key = (w.memref, w.offset, str(w.ap), str(w.dtype),
                       ins.tile_position, ins.tile_size, ins.perf_mode)
                si = ins.sync_info
                clean = si is None or (not si.on_wait and not si.on_update)
                if key == last and clean:
                    n += 1
                    continue
                last = key
            keep.append(ins)
        if len(keep) != len(bb.instructions):
            bb.instructions = keep
    print(f"deduped {n} weight loads", file=sys.stderr)


def prepare_in_maps(q, k, v, cost_mat, mix1_weight, mix1_bias, mix2_weight, mix2_bias):
    q = np.asarray(q, np.float32)
    k = np.asarray(k, np.float32)
    v = np.asarray(v, np.float32)
    cost_mat = np.asarray(cost_mat, np.float32)
    mix1_weight = np.asarray(mix1_weight, np.float32)
    mix1_bias = np.asarray(mix1_bias, np.float32)
    mix2_weight = np.asarray(mix2_weight, np.float32)
    mix2_bias = np.asarray(mix2_bias, np.float32)

    in_maps = []
    for core in range(NCORES):
        b = core // 2
        h0 = (core % 2) * HPC
        qT = (np.ascontiguousarray(q[b, h0 : h0 + HPC].transpose(2, 0, 1)) * 0.25).astype(np.float16)
        kT = np.ascontiguousarray(k[b, h0 : h0 + HPC].transpose(2, 0, 1)).astype(np.float16)
        costT = np.ascontiguousarray(cost_mat[b].T.reshape(8, 64, R).transpose(1, 0, 2)).astype(np.float16)
        vv = v[b, h0 : h0 + HPC]  # (HPC, C, D)
        vxa = np.empty((128, HPC, 4, 17), np.float32)
        vxa[:, :, :, :16] = vv.reshape(HPC, 4, 128, 16).transpose(2, 0, 1, 3)
        vxa[:, :, :, 16] = 1.0
        vxa = vxa.astype(np.float16)

        w1 = mix1_weight[h0 : h0 + HPC]  # (HPC, 2, M)
        b1 = mix1_bias[h0 : h0 + HPC]  # (HPC, M)
        w2 = mix2_weight[h0 : h0 + HPC, :, 0]  # (HPC, M)
        aw = np.abs(w2)
        sg = np.sign(w2).astype(np.float32)
        A = (w1[:, 0, :] * aw).astype(np.float32)  # (HPC, M)
        Cc = (w1[:, 1, :] * aw).astype(np.float32)
        Bb = (b1 * aw).astype(np.float32)

        w1s = np.zeros((128, HPC, 8, 128), np.float32)
        for g in range(8):
            for c8 in range(8):
                cols = slice(c8 * 16, c8 * 16 + 16)
                w1s[8 * g + c8, :, g, cols] = A
                w1s[64 + 8 * g + c8, :, g, cols] = Cc
        w1s = w1s.astype(np.float16)
        # w2s[16 c8 + m, h, g, jj, 64 jj + 8 g + c8] = sign(w2_m):
        # DR k-tile jj of group g feeds chunk column 64 jj + 8 g + c8 of the
        # (128 c, 512 r) mixed output.
        w2s = np.zeros((128, HPC, 8, 2, 128), np.float32)
        for g in range(8):
            for jj in range(2):
                for c8 in range(8):
                    w2s[c8 * 16 : c8 * 16 + 16, :, g, jj, 64 * jj + 8 * g + c8] = sg.T
        w2s = w2s.astype(ml_dtypes.float8_e4m3)
        bvs = np.tile(Bb.T, (8, 1)).astype(np.float32)  # (128, HPC)

        in_maps.append(
            dict(qT=qT, kT=kT, costT=costT, vxp=vxa, w1s=w1s, w2s=w2s, bvs=bvs)
        )
    return in_maps


def assemble(results):
    full = np.empty((B, R, H * D), np.float32)
    for core in range(NCORES):
        b = core // 2
        c0 = (core % 2) * HPC * D
        o = results[core]["out"]  # (HPC, D+1, R); row D is the softmax denom
        o = o[:, :D, :] / o[:, D : D + 1, :]
        full[b, :, c0 : c0 + HPC * D] = o.transpose(2, 0, 1).reshape(R, HPC * D)
    return full


_nc_cache = None


def _install_ntff_hook():
    """The agent image's antenv lacks axon_hooks; recreate it and register
    the ctypes NTFF profiling hook so trace=True yields exec times."""
    import types

    try:
        import antenv

        try:
            import antenv.axon_hooks  # noqa: F401

            return
        except ImportError:
            pass
        mod = types.ModuleType("antenv.axon_hooks")
        mod._hook = None
        mod.set_axon_ntff_profile_hook = lambda h: setattr(mod, "_hook", h)
        mod.get_axon_ntff_profile_hook = lambda: mod._hook
        sys.modules["antenv.axon_hooks"] = mod
        antenv.axon_hooks = mod
        from trn_agent_boot.trn_boot import _ntff_profile_via_ctypes

        mod._hook = _ntff_profile_via_ctypes("/opt/axon/libaxon_pjrt.so")
    except Exception as e:  # profiling is best-effort
        print(f"ntff hook install failed: {e}", file=sys.stderr)


def kernel(**inputs) -> np.ndarray:
    global _nc_cache, last_results
    if _nc_cache is None:
        _nc_cache = build_bass()
    in_maps = prepare_in_maps(**inputs)
    trace = bool(int(os.environ.get("KERNEL_TRACE", "0")))
    if trace:
        _install_ntff_hook()
        import concourse.bass_utils as bu

        bu.upload_artifacts = lambda tmpdir: f"local:{tmpdir}"
    res = run_bass_kernel_spmd(_nc_cache, in_maps, list(range(NCORES)), trace=trace)
    last_results = res
    return assemble(res.results)

